# revision 16
# baseline (speedup 1.0000x reference)
"""Trainium2 Bass kernel for nn_CosineLoss (data-parallel over 8 NeuronCores).

loss = -sum_n pred[n, t[n]] / (||pred[n]|| + eps) / N
       + 0.1 * mean_n (1 - ||pred[n]||)^2

Strategy per core (8192 rows x 1000 cols, f32):
  - Stream [128, 8*1000] super-tiles from HBM (4 MB per HWDGE dma_start).
  - ACT engine: Square activation with accum_out -> per-row sum of squares.
  - GpSimd ap_gather: per 16-partition group, gather the 8 blocks' target
    columns; extract the per-partition diagonal with tiny DVE
    tensor_mul + tensor_reduce ops against a precomputed (i%16 == p%16) mask.
  - Final: sqrt / +eps / reciprocal / weighted reduces -> per-partition
    partial sums [128, 2]; host sums 8*128 partials into the scalar.
"""

import sys

for _p in ("/root/.axon_site/_ro/trn_rl_repo", "/opt/trn_rl_repo"):
    if _p not in sys.path:
        sys.path.append(_p)

import numpy as np

N = 65536
C = 1000
NCORES = 8
R = N // NCORES          # rows per core
P = 128                  # partitions
NT = R // P              # 64 row-blocks per core
SUP = 8                  # row-blocks per super-tile
NSUP = NT // SUP         # 8 super-tiles per core
EPS = 1e-9
NORM_FACTOR = 0.1
# Per super-tile: blocks 0..NA-1 run on ACT (Square + accum), blocks
# NA..SUP-1 run on DVE via bn_stats (one-pass mean/var -> sumsq fixup).
NA = 4

_STATE = {}


def _build_program():
    import concourse.bacc as bacc
    import concourse.bass as bass
    import concourse.mybir as mybir
    import concourse.tile as tile
    from concourse._compat import with_exitstack

    f32 = mybir.dt.float32
    i16 = mybir.dt.int16
    AF = mybir.ActivationFunctionType
    ALU = mybir.AluOpType

    nc = bacc.Bacc(
        "TRN2",
        target_bir_lowering=False,
        debug=False,
        enable_asserts=False,
        num_devices=NCORES,
    )

    pred_d = nc.dram_tensor("pred", [R, C], f32, kind="ExternalInput").ap()
    tgt_d = nc.dram_tensor("tgt", [P, NT], i16, kind="ExternalInput").ap()
    m128_d = nc.dram_tensor("m128", [P, SUP * 16], f32, kind="ExternalInput").ap()
    out_d = nc.dram_tensor("out", [P, 2], f32, kind="ExternalOutput").ap()

    # [R, C] viewed as [p, block, c]: row = block*128 + p
    pred_v = pred_d.rearrange("(sb p) c -> p sb c", p=P)

    with tile.TileContext(nc) as tc:
        from contextlib import ExitStack

        with ExitStack() as ctx:
            data_pool = ctx.enter_context(tc.tile_pool(name="data", bufs=4))
            g16_pool = ctx.enter_context(tc.tile_pool(name="g16", bufs=2))
            scr_pool = ctx.enter_context(tc.tile_pool(name="scr", bufs=2))
            junk_pool = ctx.enter_context(tc.tile_pool(name="junk", bufs=2))
            persist = ctx.enter_context(tc.tile_pool(name="persist", bufs=1))

            tgt_t = persist.tile([P, NT], i16)
            nc.sync.dma_start(tgt_t[:], tgt_d[:])
            m128_t = persist.tile([P, SUP * 16], f32)
            nc.sync.dma_start(m128_t[:], m128_d[:])

            # Preload the sqrt_and_others ACT table set while ACT is idle;
            # Square is a filler in every set, so no mid-kernel set switch.
            dummy = persist.tile([P, 1], f32)
            nc.gpsimd.memset(dummy[:], 1.0)
            dummy2 = persist.tile([P, 1], f32)
            nc.scalar.activation(dummy2[:], dummy[:], AF.Sqrt)

            sumsq = persist.tile([P, NT], f32)
            gath = persist.tile([P, NT], f32)
            # bn_stats outputs for the NBN bn-blocks of each super-tile:
            # 2 chunks x 6 stats per block.
            NBN = SUP - NA
            stats_t = persist.tile([P, NSUP * NBN * 12], f32)

            # Final-phase tiles (written in two column phases).
            norms = persist.tile([P, NT], f32)
            denom = persist.tile([P, NT], f32)
            inv = persist.tile([P, NT], f32)
            junk64 = persist.tile([P, NT], f32)
            nlj = persist.tile([P, NT], f32)
            gpart = persist.tile([P, 2], f32)
            npart = persist.tile([P, 2], f32)
            out_t = persist.tile([P, 2], f32)

            def emit_fixup(jb0, jb1, ss_view):
                n = jb1 - jb0
                sv = stats_t[:, 12 * jb0 : 12 * jb1].rearrange(
                    "p (j k) -> p j k", k=12
                )

                def col(k):
                    return sv[:, :, k : k + 1]

                fx = junk_pool.tile([P, 6 * n], f32, tag=f"fx{jb0}")
                f0, f1, f2 = fx[:, 0:n], fx[:, n : 2 * n], fx[:, 2 * n : 3 * n]
                f3, f4, f5 = (
                    fx[:, 3 * n : 4 * n],
                    fx[:, 4 * n : 5 * n],
                    fx[:, 5 * n : 6 * n],
                )
                nc.vector.tensor_mul(f0[:], col(1), col(1))
                nc.vector.tensor_mul(f1[:], col(4), col(4))
                nc.vector.tensor_add(f2[:], f0[:], f1[:])
                nc.vector.tensor_mul(f0[:], col(7), col(7))
                nc.vector.tensor_mul(f1[:], col(10), col(10))
                nc.vector.tensor_add(f3[:], f0[:], f1[:])
                nc.vector.tensor_add(f4[:], f2[:], f3[:])  # sum mean^2
                nc.vector.tensor_add(f0[:], col(2), col(5))
                nc.vector.tensor_add(f1[:], col(8), col(11))
                nc.vector.tensor_add(f2[:], f0[:], f1[:])  # sum M2
                nc.vector.tensor_scalar_mul(f5[:], f4[:], float(C // 4))
                nc.vector.tensor_add(ss_view, f5[:], f2[:])

            def emit_final(c0, c1, phase):
                nc.scalar.activation(
                    norms[:, c0:c1], sumsq[:, c0:c1], AF.Sqrt
                )
                nc.vector.tensor_scalar_add(denom[:, c0:c1], norms[:, c0:c1], EPS)
                nc.vector.reciprocal(inv[:, c0:c1], denom[:, c0:c1])
                nc.vector.tensor_mul(
                    junk64[:, c0:c1], gath[:, c0:c1], inv[:, c0:c1]
                )
                nc.vector.tensor_reduce(
                    gpart[:, phase : phase + 1],
                    junk64[:, c0:c1],
                    mybir.AxisListType.X,
                    ALU.add,
                )
                nc.scalar.activation(
                    nlj[:, c0:c1],
                    norms[:, c0:c1],
                    AF.Square,
                    bias=1.0,
                    scale=-1.0,
                    accum_out=npart[:, phase : phase + 1],
                )

            for s in range(NSUP):
                data = data_pool.tile([P, SUP * C], f32)
                if s in (0, NSUP - 1):
                    # Split first/last super-tiles into per-block DMAs (same
                    # SP HWDGE ring, FIFO order) so the first blocks land
                    # ~4us in and the last blocks drain compute sooner.
                    for b in range(SUP):
                        nc.sync.dma_start(
                            data[:, bass.ts(b, C)], pred_v[:, s * SUP + b, :]
                        )
                else:
                    nc.sync.dma_start(data[:], pred_v[:, bass.ts(s, SUP), :])

                g16 = g16_pool.tile([P, SUP * 16], f32)
                nc.gpsimd.ap_gather(
                    g16[:],
                    data[:],
                    tgt_t[:, bass.ts(s, SUP)],
                    channels=P,
                    num_elems=SUP * C,
                    d=1,
                    num_idxs=SUP * 16,
                )

                for b in range(SUP):
                    j = s * SUP + b
                    if b < NA:
                        # A-block: ACT Square with accumulate.
                        scrA = scr_pool.tile([P, C], f32, tag="scrA")
                        nc.scalar.activation(
                            scrA[:],
                            data[:, bass.ts(b, C)],
                            AF.Square,
                            accum_out=sumsq[:, j : j + 1],
                        )
                    else:
                        # bn-block: one-pass mean/var stats on DVE.
                        jb = NBN * s + (b - NA)
                        nc.vector.bn_stats(
                            stats_t[:, 12 * jb : 12 * jb + 6],
                            data[:, 2 * 500 * b : 2 * 500 * b + 500],
                        )
                        nc.vector.bn_stats(
                            stats_t[:, 12 * jb + 6 : 12 * jb + 12],
                            data[:, 2 * 500 * b + 500 : 2 * 500 * b + 1000],
                        )

                gm = junk_pool.tile([P, SUP * 16], f32)
                nc.vector.tensor_mul(gm[:], g16[:], m128_t[:])
                nc.vector.tensor_reduce(
                    gath[:, bass.ts(s, SUP)],
                    gm[:].rearrange("p (b i) -> p b i", i=16),
                    mybir.AxisListType.X,
                    ALU.add,
                )

                if s == NSUP - 2:
                    # Phase 1: fixup + final for super-tiles 0..NSUP-2 while
                    # the last super-tile streams in.
                    ss_v1 = sumsq[:].rearrange("p (s b) -> p s b", b=SUP)[
                        :, 0 : NSUP - 1, NA:SUP
                    ]
                    emit_fixup(0, NBN * (NSUP - 1), ss_v1)
                    emit_final(0, SUP * (NSUP - 1), 0)

            # Phase 2: last super-tile only.
            ss_v2 = sumsq[:].rearrange("p (s b) -> p s b", b=SUP)[
                :, NSUP - 1 : NSUP, NA:SUP
            ]
            emit_fixup(NBN * (NSUP - 1), NBN * NSUP, ss_v2)
            emit_final(SUP * (NSUP - 1), NT, 1)

            nc.vector.tensor_add(out_t[:, 0:1], gpart[:, 0:1], gpart[:, 1:2])
            nc.vector.tensor_add(out_t[:, 1:2], npart[:, 0:1], npart[:, 1:2])
            nc.sync.dma_start(out_d[:], out_t[:])

    nc.compile()
    return nc


def _host_shard(prediction, target):
    """Build per-core input maps."""
    prediction = np.asarray(prediction, dtype=np.float32)
    target = np.asarray(target)

    m128 = (
        (np.arange(SUP * 16)[None, :] % 16) == (np.arange(P)[:, None] % 16)
    ).astype(np.float32)

    in_maps = []
    for k in range(NCORES):
        pred_k = np.ascontiguousarray(prediction[k * R : (k + 1) * R])
        t_k = target[k * R : (k + 1) * R].astype(np.int64)
        # tgt[p, j] = (j % SUP) * C + target[k*R + 128*j + p]
        tk = t_k.reshape(NT, P).T  # [128, 64]
        off = (np.arange(NT) % SUP) * C  # [64]
        tgt_k = (tk + off[None, :]).astype(np.int16)
        in_maps.append({"pred": pred_k, "tgt": tgt_k, "m128": m128})
    return in_maps


def _combine(results):
    """results: list of {'out': [128, 2]} per core -> scalar f32 loss."""
    outs = np.stack([np.asarray(r["out"], dtype=np.float64) for r in results])
    G = outs[:, :, 0].sum()
    NL = outs[:, :, 1].sum()
    loss = -G / N + NORM_FACTOR * (NL / N)
    return np.float32(loss)


def get_nc():
    if "nc" not in _STATE:
        _STATE["nc"] = _build_program()
    return _STATE["nc"]


def kernel(prediction, target):
    from concourse.bass_utils import run_bass_kernel_spmd

    nc = get_nc()
    in_maps = _host_shard(prediction, target)
    res = run_bass_kernel_spmd(nc, in_maps, list(range(NCORES)))
    return _combine(res.results)
